# revision 5
# baseline (speedup 1.0000x reference)
"""Trainium2 Bass kernel for CrossAttentionConditionInjection.

Math note: in the reference, K and V are projections of a single per-batch
condition vector broadcast identically across all S key positions.  The
attention scores are therefore constant along the softmax axis, softmax is
exactly uniform (1/S each), and the attention output is the mean of S
identical V rows, i.e. V itself.  The whole module collapses exactly to

    out[b, s, :] = (condition[b] @ Wv.T + bv) @ Wo.T + bo      (for every s)

independent of hidden_states / Wq / bq / Wk / bk.  (S = 1024 is a power of
two, so even the fp32 softmax-average path is bit-exact against this.)

Device strategy (8 NeuronCores, SPMD):
  - shard the output channel dim D=2048 into 8 slices of 256
  - each core:  vT = (Wv.T).T-chunks @ cT   (full Wv.T, K=2048)
                r  = vT.T @ Wo.T[:, shard]  (4 x 256, K=2048)
                fold bo + broadcast r across 128 partitions with a selector
                matmul, then DMA the (4, 1024, 256) output slice
  - host concatenates the 8 slices along the channel axis (layout only).
"""

import numpy as np

import concourse.bass as bass
import concourse.mybir as mybir
import concourse.tile as tile
from concourse import bacc
from concourse.bass_utils import run_bass_kernel_spmd

B = 4
S = 1024
D = 2048
N_CORES = 8
JC = D // N_CORES  # 256 output channels per core
P = 128
KT = D // P  # 16 k-chunks
GT = JC // P  # 2 j-chunks per core (unused in selector path)
FP = mybir.dt.float32


def build_nc():
    nc = bacc.Bacc(
        "TRN2",
        target_bir_lowering=False,
        debug=False,
        enable_asserts=False,
        num_devices=N_CORES,
    )

    ct_d = nc.dram_tensor("ct", [D, B], FP, kind="ExternalInput").ap()
    wvT_d = nc.dram_tensor("wvT", [D, D], FP, kind="ExternalInput").ap()
    woT_d = nc.dram_tensor("woT", [D, JC], FP, kind="ExternalInput").ap()
    bv_d = nc.dram_tensor("bv", [D], FP, kind="ExternalInput").ap()
    bo_d = nc.dram_tensor("bo", [1, JC], FP, kind="ExternalInput").ap()
    sel_d = nc.dram_tensor("sel", [B + 1, B * P], FP, kind="ExternalInput").ap()
    out_d = nc.dram_tensor("out", [B, S, JC], FP, kind="ExternalOutput").ap()

    with tile.TileContext(nc) as tc:
        with (
            tc.tile_pool(name="weights", bufs=1) as wpool,
            tc.tile_pool(name="work", bufs=1) as work,
            tc.tile_pool(name="pv", bufs=2, space="PSUM") as pv_pool,
            tc.tile_pool(name="pr", bufs=1, space="PSUM") as pr_pool,
            tc.tile_pool(name="pb", bufs=2, space="PSUM") as pb_pool,
        ):
            wv_sb = wpool.tile([P, KT, D], FP)
            wo_sb = wpool.tile([P, KT, JC], FP)
            ct_sb = work.tile([P, KT, B], FP)
            bv_sb = work.tile([P, KT], FP)
            vt_sb = work.tile([P, KT, B], FP)
            rb_sb = work.tile([B + 1, JC], FP)
            sel_sb = work.tile([B + 1, B * P], FP)
            bc_sb = work.tile([P, B, JC], FP)

            # ---- loads ----
            for t in range(KT):
                nc.sync.dma_start(wv_sb[:, t, :], wvT_d[t * P : (t + 1) * P, :])
            nc.sync.dma_start(
                wo_sb[:, :, :], woT_d.rearrange("(g p) j -> p g j", p=P)
            )
            nc.sync.dma_start(
                ct_sb[:, :, :], ct_d.rearrange("(t p) b -> p t b", p=P)
            )
            nc.sync.dma_start(bv_sb[:, :], bv_d.rearrange("(t p) -> p t", p=P))
            nc.sync.dma_start(rb_sb[B : B + 1, :], bo_d[:, :])

            # selector: sel[k, b*128+p] = 1 if k == b or k == B else 0
            nc.sync.dma_start(sel_sb[:, :], sel_d[:, :])

            # ---- mm1: vT[d, b] = sum_k WvT[k, d] * cT[k, b]  (+ bv) ----
            for g in range(KT):
                pv = pv_pool.tile([P, B], FP)
                for t in range(KT):
                    nc.tensor.matmul(
                        pv[:, :],
                        wv_sb[:, t, g * P : (g + 1) * P],
                        ct_sb[:, t, :],
                        start=(t == 0),
                        stop=(t == KT - 1),
                    )
                nc.vector.tensor_scalar_add(
                    vt_sb[:, g, :], pv[:, :], bv_sb[:, g : g + 1]
                )

            # ---- mm2: r[b, j] = sum_d vT[d, b] * WoT[d, j] ----
            pr = pr_pool.tile([B, JC], FP)
            for g in range(KT):
                nc.tensor.matmul(
                    pr[:, :],
                    vt_sb[:, g, :],
                    wo_sb[:, g, :],
                    start=(g == 0),
                    stop=(g == KT - 1),
                )
            nc.vector.tensor_copy(rb_sb[0:B, :], pr[:, :])

            # ---- broadcast (+ bo): bc[p, j] = r[b, j] + bo[j] for all p ----
            for b in range(B):
                pb = pb_pool.tile([P, JC], FP)
                nc.tensor.matmul(
                    pb[:, :],
                    sel_sb[:, b * P : (b + 1) * P],
                    rb_sb[:, :],
                    start=True,
                    stop=True,
                )
                nc.vector.tensor_copy(bc_sb[:, b, :], pb[:, :])

            # ---- store: out[b, sc*128 + p, :] = bc[p, b, :] ----
            for b in range(B):
                for sc in range(S // P):
                    nc.sync.dma_start(
                        out_d[b, sc * P : (sc + 1) * P, :], bc_sb[:, b, :]
                    )

    nc.compile()
    return nc


def make_in_maps(hidden_states, condition, Wq, bq, Wk, bk, Wv, bv, Wo, bo):
    ct = np.ascontiguousarray(np.asarray(condition, dtype=np.float32).T)
    wvT = np.ascontiguousarray(np.asarray(Wv, dtype=np.float32).T)
    woT = np.asarray(Wo, dtype=np.float32).T
    bv = np.ascontiguousarray(np.asarray(bv, dtype=np.float32))
    bo = np.asarray(bo, dtype=np.float32)
    sel = np.zeros((B + 1, B * P), dtype=np.float32)
    for b in range(B):
        sel[b, b * P : (b + 1) * P] = 1.0
    sel[B, :] = 1.0
    in_maps = []
    for i in range(N_CORES):
        sl = slice(i * JC, (i + 1) * JC)
        in_maps.append(
            {
                "ct": ct,
                "wvT": wvT,
                "woT": np.ascontiguousarray(woT[:, sl]),
                "bv": bv,
                "bo": np.ascontiguousarray(bo[sl]).reshape(1, JC),
                "sel": sel,
            }
        )
    return in_maps


_NC_CACHE = None


def get_nc():
    global _NC_CACHE
    if _NC_CACHE is None:
        _NC_CACHE = build_nc()
    return _NC_CACHE


def kernel(**inputs):
    nc = get_nc()
    in_maps = make_in_maps(**inputs)
    res = run_bass_kernel_spmd(nc, in_maps, core_ids=list(range(N_CORES)))
    out = np.concatenate([r["out"] for r in res.results], axis=-1)
    return out


# revision 11
# speedup vs baseline: 1.7551x; 1.7551x over previous
"""Trainium2 Bass kernel for CrossAttentionConditionInjection.

Math note: in the reference, K and V are projections of a single per-batch
condition vector broadcast identically across all S key positions.  The
attention scores are therefore constant along the softmax axis, softmax is
exactly uniform (1/S each), and the attention output is the mean of S
identical V rows, i.e. V itself.  The whole module collapses exactly to

    out[b, s, :] = (condition[b] @ Wv.T + bv) @ Wo.T + bo      (for every s)

independent of hidden_states / Wq / bq / Wk / bk.  (S = 1024 is a power of
two, so even the fp32 softmax-average path is bit-exact against this.)

Device strategy (8 NeuronCores on one trn2 chip, SPMD, one NEFF):
  - Wv.T is column-sharded 8x: core i computes v[:, 256i:256(i+1)],
    transposes it on the PE, adds its bv shard, and AllGathers the
    (256, 4) vT shards into the full (2048, 4) vT.
  - Wo.T is column-sharded 8x: core i computes r[:, 256i:256(i+1)] =
    vT.T @ Wo.T[:, shard]  (full K=2048 contraction).
  - bo + the broadcast of r across sequence positions are folded into a
    single selector matmul per batch entry; each core writes its
    (4, 1024, 256) output slice; the host concatenates (layout only).
"""

import numpy as np

import concourse.bass as bass
import concourse.mybir as mybir
import concourse.tile as tile
from concourse import bacc
from concourse.bass_utils import run_bass_kernel_spmd
from concourse.masks import make_identity

B = 4
S = 1024
D = 2048
N_CORES = 8
JC = D // N_CORES  # 256 channels per core (both v-shard and out-shard)
P = 128
KT = D // P  # 16 k-chunks
FP = mybir.dt.float32
FPR = mybir.dt.float32r

USE_F32R = False


def build_nc():
    nc = bacc.Bacc(
        "TRN2",
        target_bir_lowering=False,
        debug=False,
        enable_asserts=False,
        num_devices=N_CORES,
    )

    ct_d = nc.dram_tensor("ct", [D, B], FP, kind="ExternalInput").ap()
    wv_d = nc.dram_tensor("wv_s", [D, JC], FP, kind="ExternalInput").ap()
    wo_d = nc.dram_tensor("wo_s", [D, JC], FP, kind="ExternalInput").ap()
    bv_d = nc.dram_tensor("bv_s", [JC], FP, kind="ExternalInput").ap()
    bo_d = nc.dram_tensor("bo_s", [1, JC], FP, kind="ExternalInput").ap()
    sel_d = nc.dram_tensor("sel", [B + 1, B * P], FP, kind="ExternalInput").ap()
    out_d = nc.dram_tensor("out", [B, S, JC], FP, kind="ExternalOutput").ap()

    def mmdt(ap):
        return ap.bitcast(FPR) if USE_F32R else ap

    with tile.TileContext(nc) as tc:
        with (
            tc.tile_pool(name="weights", bufs=1) as wpool,
            tc.tile_pool(name="work", bufs=1) as work,
            tc.tile_pool(name="dram", bufs=1, space="DRAM") as dram,
            tc.tile_pool(name="pv", bufs=2, space="PSUM") as pv_pool,
            tc.tile_pool(name="pt", bufs=2, space="PSUM") as pt_pool,
            tc.tile_pool(name="pr", bufs=1, space="PSUM") as pr_pool,
            tc.tile_pool(name="pb", bufs=2, space="PSUM") as pb_pool,
        ):
            wv_sb = wpool.tile([P, KT, JC], FP)
            wo_sb = wpool.tile([P, KT, JC], FP)
            ct_sb = work.tile([P, KT, B], FP)
            bv_sb = work.tile([P, JC // P], FP)
            vl_sb = work.tile([B, JC], FP)  # local v shard (b, 256)
            vtl_sb = work.tile([P, JC // P, B], FP)  # local vT shard
            vt_sb = work.tile([P, KT, B], FP)  # gathered full vT
            rb_sb = work.tile([B + 1, JC], FP)
            sel_sb = work.tile([B + 1, B * P], FP)
            bc_sb = work.tile([P, B, JC], FP)
            id4_sb = work.tile([B, B], FP)
            make_identity(nc, id4_sb[:, :])

            vt_gin = dram.tile([JC, B], FP)  # all-gather input bounce
            vt_gout = dram.tile([D, B], FP)  # all-gather output bounce

            # ---- loads ----
            nc.sync.dma_start(wv_sb[:, :, :], wv_d.rearrange("(t p) j -> p t j", p=P))
            nc.sync.dma_start(wo_sb[:, :, :], wo_d.rearrange("(g p) j -> p g j", p=P))
            nc.sync.dma_start(ct_sb[:, :, :], ct_d.rearrange("(t p) b -> p t b", p=P))
            nc.sync.dma_start(bv_sb[:, :], bv_d.rearrange("(g p) -> p g", p=P))
            nc.sync.dma_start(rb_sb[B : B + 1, :], bo_d[:, :])
            nc.sync.dma_start(sel_sb[:, :], sel_d[:, :])

            # ---- mm1: vl[b, j] = sum_k cT[k, b] * WvT_shard[k, j] ----
            pv = pv_pool.tile([B, JC], FP)
            for t in range(KT):
                nc.tensor.matmul(
                    pv[:, :],
                    mmdt(ct_sb[:, t, :]),
                    mmdt(wv_sb[:, t, :]),
                    start=(t == 0),
                    stop=(t == KT - 1),
                )
            nc.vector.tensor_copy(vl_sb[:, :], pv[:, :])

            # ---- transpose local v shard, add bv shard ----
            for g in range(JC // P):
                pt = pt_pool.tile([P, B], FP)
                nc.tensor.transpose(
                    pt[:, :], vl_sb[:, g * P : (g + 1) * P], id4_sb[:, :]
                )
                nc.vector.tensor_scalar_add(
                    vtl_sb[:, g, :], pt[:, :], bv_sb[:, g : g + 1]
                )

            # ---- all-gather vT shards across the 8 cores ----
            nc.sync.dma_start(
                vt_gin.rearrange("(g p) b -> p g b", p=P), vtl_sb[:, :, :]
            )
            nc.gpsimd.collective_compute(
                "AllGather",
                mybir.AluOpType.bypass,
                replica_groups=[list(range(N_CORES))],
                ins=[vt_gin[:, :].opt()],
                outs=[vt_gout[:, :].opt()],
            )
            nc.sync.dma_start(
                vt_sb[:, :, :], vt_gout.rearrange("(g p) b -> p g b", p=P)
            )

            # ---- mm2: r[b, j] = sum_d vT[d, b] * WoT_shard[d, j] ----
            pr = pr_pool.tile([B, JC], FP)
            for g in range(KT):
                nc.tensor.matmul(
                    pr[:, :],
                    mmdt(vt_sb[:, g, :]),
                    mmdt(wo_sb[:, g, :]),
                    start=(g == 0),
                    stop=(g == KT - 1),
                )
            nc.vector.tensor_copy(rb_sb[0:B, :], pr[:, :])

            # ---- broadcast (+ bo): bc[p, j] = r[b, j] + bo[j] for all p ----
            for b in range(B):
                pb = pb_pool.tile([P, JC], FP)
                nc.tensor.matmul(
                    pb[:, :],
                    mmdt(sel_sb[:, b * P : (b + 1) * P]),
                    mmdt(rb_sb[:, :]),
                    start=True,
                    stop=True,
                )
                nc.vector.tensor_copy(bc_sb[:, b, :], pb[:, :])

            # ---- store: out[b, sc*128 + p, :] = bc[p, b, :] ----
            for b in range(B):
                for sc in range(S // P):
                    nc.sync.dma_start(
                        out_d[b, sc * P : (sc + 1) * P, :], bc_sb[:, b, :]
                    )

    nc.compile()
    return nc


def make_in_maps(hidden_states, condition, Wq, bq, Wk, bk, Wv, bv, Wo, bo):
    ct = np.ascontiguousarray(np.asarray(condition, dtype=np.float32).T)
    wvT = np.asarray(Wv, dtype=np.float32).T
    woT = np.asarray(Wo, dtype=np.float32).T
    bv = np.asarray(bv, dtype=np.float32)
    bo = np.asarray(bo, dtype=np.float32)
    sel = np.zeros((B + 1, B * P), dtype=np.float32)
    for b in range(B):
        sel[b, b * P : (b + 1) * P] = 1.0
    sel[B, :] = 1.0
    in_maps = []
    for i in range(N_CORES):
        sl = slice(i * JC, (i + 1) * JC)
        in_maps.append(
            {
                "ct": ct,
                "wv_s": np.ascontiguousarray(wvT[:, sl]),
                "wo_s": np.ascontiguousarray(woT[:, sl]),
                "bv_s": np.ascontiguousarray(bv[sl]),
                "bo_s": np.ascontiguousarray(bo[sl]).reshape(1, JC),
                "sel": sel,
            }
        )
    return in_maps


_NC_CACHE = None


def get_nc():
    global _NC_CACHE
    if _NC_CACHE is None:
        _NC_CACHE = build_nc()
    return _NC_CACHE


def kernel(**inputs):
    nc = get_nc()
    in_maps = make_in_maps(**inputs)
    res = run_bass_kernel_spmd(nc, in_maps, core_ids=list(range(N_CORES)))
    out = np.concatenate([r["out"] for r in res.results], axis=-1)
    return out
